# revision 1
# baseline (speedup 1.0000x reference)
"""KAARMA-style multi-cell kernel recurrence on 8 Trainium2 NeuronCores.

Math reformulation (validated vs reference to ~1e-6 rel):
  per step t, per batch b, for every dictionary atom (c, m) [cm = 800 atoms]:
    phi[b,cm]  = exp(-|s_b - S_cm|^2 - (x_tb - U_cm)^2)
    gate[b,c]  = softmax_c(MLP(x_tb))               (precomputable, x-only)
    s'_b       = sum_cm gate[b,cell(cm)] * phi[b,cm] * A[cm,:]
  Expand the squares and fold the gate into the exponent:
    psi[b,cm]  = exp( 2 s_b.S_cm - |s_b|^2 + 2 x U_cm - x^2 + logsoftmax_c )
    s'_b       = sum_cm psi[b,cm] * (A[cm,:] * exp(-|S_cm|^2 - U_cm^2))
  so one step = matmul([K,B] -> [800,B]) -> exp -> matmul([800,B] -> [16,B]).

Contraction-row layout (engine SBUF writes must start at partition 0/32/64/96,
so segments are 32-aligned and the gaps carry zero weights):
  rows  0:16  s            (weights 2*S)
  rows 32:48  s^2          (weights -1)
  row  64     x_t          (weights 2*U)
  rows 65:73  one-hot cell (data = lg[b,c] - x_t^2)
K = 73; gap rows 16:32, 48:64 are zeroed once and have zero weight columns.

Sharding: pure data parallel, batch 512 -> 64 per core on 8 cores.
Device layout is feature-major: state sT [16, B_local] in PSUM, psi chunks
[128, B_local] (7 chunks of 128 atoms, cm padded 800 -> 896).
"""

import numpy as np

N_CORES = 8
CM = 800
CM_PAD = 896
NCHUNK = 7
KROWS = 73
NSTATE = 16
STAGE = 64  # recurrence steps per staging DMA

_PROGRAM_CACHE = {}


def _build_program(B_local, T, rep=1):
    import concourse.bass as bass
    import concourse.bacc as bacc
    import concourse.tile as tile
    from concourse import mybir
    from contextlib import ExitStack

    f32 = mybir.dt.float32
    Act = mybir.ActivationFunctionType

    # Bacc (not Bass): its compile() runs generate_event_semaphores, which
    # splits multi-wait instructions (TRN2 allows 1 wait per instruction)
    nc = bacc.Bacc("TRN2", target_bir_lowering=False, debug=False)
    W_d = nc.dram_tensor("Wk", [KROWS, CM_PAD], f32, kind="ExternalInput")
    A2_d = nc.dram_tensor("A2e", [CM_PAD, NSTATE], f32, kind="ExternalInput")
    R_d = nc.dram_tensor("Rt", [9, T * B_local], f32, kind="ExternalInput")
    O_d = nc.dram_tensor("O1", [1, T * B_local], f32, kind="ExternalOutput")

    with tile.TileContext(nc) as tc, ExitStack() as ctx:
        singles = ctx.enter_context(tc.tile_pool(name="singles", bufs=1))
        rpool = ctx.enter_context(tc.tile_pool(name="rstage", bufs=2))
        opool = ctx.enter_context(tc.tile_pool(name="ostage", bufs=2))
        psipool = ctx.enter_context(tc.tile_pool(name="psi", bufs=4))
        rhspool = ctx.enter_context(tc.tile_pool(name="rhs", bufs=6))
        apsum1 = ctx.enter_context(tc.tile_pool(name="apsum1", bufs=2, space="PSUM"))
        apsum2 = ctx.enter_context(tc.tile_pool(name="apsum2", bufs=2, space="PSUM"))
        spsum = ctx.enter_context(tc.tile_pool(name="spsum", bufs=4, space="PSUM"))

        Wsb = singles.tile([KROWS, CM_PAD], f32)
        nc.sync.dma_start(out=Wsb, in_=W_d[:, :])
        A2sb = singles.tile([128, NCHUNK, NSTATE], f32)
        nc.sync.dma_start(out=A2sb, in_=A2_d.rearrange("(c p) n -> p c n", p=128))

        # two independent half-batches per core, software-pipelined so one
        # half's matmuls hide the other half's state-handoff tail
        BH = B_local // 2
        NSTEP = rep * T

        sP = [None, None]       # state psum feeding step t, per half
        rhs = [None, None]      # rhs tile for step t, per half
        rhs_next = [None, None]
        pend_r = [None, None]   # deferred stage-boundary R-copy args
        out_pend = []           # (ost_ap, sp) output rows not yet copied
        Rsts = {}
        Osts = {}

        def col(ti, h):
            return ti * B_local + h * BH

        def prep_rhs(h, t):
            """Allocate + zero + R-rows for half h's step-t rhs (gpsimd)."""
            nsi, nti = divmod(t, STAGE)
            rt = rhspool.tile([KROWS, BH], f32, tag="rhs", name=f"rhs{t}_{h}")
            nc.gpsimd.memset(rt[:, :], 0.0)
            if nsi in Rsts:
                nc.gpsimd.tensor_copy(
                    out=rt[64:KROWS, :],
                    in_=Rsts[nsi][0:9, col(nti, h) : col(nti, h) + BH],
                )
            else:
                pend_r[h] = (rt, nti)
            return rt

        for t in range(NSTEP):
            si, ti = divmod(t, STAGE)
            if ti == 0:
                Rst = rpool.tile([32, STAGE * B_local], f32, tag="rstage")
                rsi = si % (T // STAGE)
                nc.sync.dma_start(
                    out=Rst[0:9, :],
                    in_=R_d[:, rsi * STAGE * B_local : (rsi + 1) * STAGE * B_local],
                )
                Rsts[si] = Rst
                Osts[si] = opool.tile([1, STAGE * B_local], f32, tag="ostage", name=f"ost{si}")
                for h in (0, 1):
                    if pend_r[h] is not None:
                        rt, nti = pend_r[h]
                        nc.gpsimd.tensor_copy(
                            out=rt[64:KROWS, :],
                            in_=Rst[0:9, col(nti, h) : col(nti, h) + BH],
                        )
                        pend_r[h] = None

            for h in (0, 1):
                if rhs[h] is None:
                    rhs[h] = prep_rhs(h, t)
                # handoff tail on DVE alone (measured faster than a
                # parallel DVE/ACT tail: fewer cross-engine waits on mm1);
                # the square reads the fresh SBUF copy, not PSUM
                if sP[h] is not None:
                    nc.vector.tensor_scalar_add(rhs[h][0:NSTATE, :], sP[h], 0.0)
                    nc.vector.tensor_mul(
                        rhs[h][32:48, :], rhs[h][0:NSTATE, :], rhs[h][0:NSTATE, :]
                    )

            # previous step's output rows (DVE, after the tails in FIFO order)
            while out_pend:
                ap, sp = out_pend.pop()
                nc.vector.tensor_scalar_add(ap, sp[0:1, :], 0.0)
            if ti == 0 and si > 0 and (si - 1) in Osts:
                pso = (si - 1) % (T // STAGE)
                nc.sync.dma_start(
                    out=O_d[:, pso * STAGE * B_local : (pso + 1) * STAGE * B_local],
                    in_=Osts[si - 1],
                )
                del Osts[si - 1]

            G1, G2 = (0, 1, 2, 3), (4, 5, 6)
            argp = {}
            sP_new = [None, None]
            for h in (0, 1):  # mm1 for both halves first
                argp[h, 0] = apsum1.tile([128, len(G1) * BH], f32, tag="a1", name=f"a1_{t}_{h}")
                argp[h, 1] = apsum2.tile([128, len(G2) * BH], f32, tag="a2", name=f"a2_{t}_{h}")
                for g, grp in enumerate((G1, G2)):
                    for i, k in enumerate(grp):
                        nc.tensor.matmul(
                            argp[h, g][:, i * BH : (i + 1) * BH],
                            lhsT=Wsb[:, k * 128 : (k + 1) * 128],
                            rhs=rhs[h],
                            start=True,
                            stop=True,
                        )
            psis = {}
            for h in (0, 1):
                for g, grp in enumerate((G1, G2)):
                    psis[h, g] = psipool.tile(
                        [128, len(grp) * BH], f32, tag="psi", name=f"psi{t}_{h}{g}"
                    )
                    nc.scalar.activation(out=psis[h, g], in_=argp[h, g], func=Act.Exp)
            for h in (0, 1):  # mm2 for both halves
                sP_new[h] = spsum.tile([NSTATE, BH], f32, tag="s", name=f"s{t}_{h}")
                for g, grp in enumerate((G1, G2)):
                    for i, k in enumerate(grp):
                        nc.tensor.matmul(
                            sP_new[h],
                            lhsT=A2sb[:, k, :],
                            rhs=psis[h, g][:, i * BH : (i + 1) * BH],
                            start=(k == 0),
                            stop=(k == NCHUNK - 1),
                            skip_group_check=True,
                        )
            for h in (0, 1):
                sP[h] = sP_new[h]
                out_pend.append(
                    (Osts[si][:, col(ti, h) : col(ti, h) + BH], sP[h])
                )
                # prefetch next step's rhs behind the matmuls
                rhs[h] = prep_rhs(h, t + 1) if t + 1 < NSTEP else None

        # final output rows + last stage flush
        while out_pend:
            ap, sp = out_pend.pop()
            nc.vector.tensor_scalar_add(ap, sp[0:1, :], 0.0)
        lsi = NSTEP // STAGE - 1
        lso = lsi % (T // STAGE)
        nc.sync.dma_start(
            out=O_d[:, lso * STAGE * B_local : (lso + 1) * STAGE * B_local],
            in_=Osts[lsi],
        )

    nc.compile()
    return nc


def _host_precompute(x, S, U, A, W1, b1, W2, b2):
    B, T = x.shape
    C, M, N = S.shape
    B_local = B // N_CORES

    # state permutation: put the output component (N-1) at row 0
    perm = np.r_[N - 1, np.arange(N - 1)]

    Sf = S.reshape(C * M, N).astype(np.float32)
    Uf = U.reshape(C * M).astype(np.float32)
    C1 = (Sf * Sf).sum(1) + Uf * Uf
    A2e = np.zeros((CM_PAD, N), np.float32)
    A2e[:CM] = (A.reshape(C * M, N) * np.exp(-C1)[:, None])[:, perm]

    Wk = np.zeros((KROWS, CM_PAD), np.float32)
    Wk[0:N, :CM] = 2.0 * Sf.T[perm]
    Wk[32:48, :CM] = -1.0
    Wk[64, :CM] = 2.0 * Uf
    for c in range(C):
        Wk[65 + c, c * M : (c + 1) * M] = 1.0

    # gate log-softmax, x-only
    h = np.maximum(x[..., None] * W1[0] + b1, 0.0)  # [B,T,16]
    g = h @ W2 + b2  # [B,T,C]
    g = g - g.max(-1, keepdims=True)
    lg = (g - np.log(np.exp(g).sum(-1, keepdims=True))).astype(np.float32)

    x2 = (x * x).astype(np.float32)
    R = np.empty((N_CORES, 9, T, B_local), np.float32)
    for i in range(N_CORES):
        bs = slice(i * B_local, (i + 1) * B_local)
        R[i, 0] = x[bs].T
        R[i, 1:] = (lg[bs] - x2[bs][..., None]).transpose(2, 1, 0)
    R = R.reshape(N_CORES, 9, T * B_local)
    return Wk, A2e, R


def kernel(x, S, U, A, W1, b1, W2, b2):
    x = np.asarray(x, np.float32)
    B, T = x.shape
    assert B % N_CORES == 0 and T % STAGE == 0
    B_local = B // N_CORES

    Wk, A2e, R = _host_precompute(
        np.asarray(x), np.asarray(S), np.asarray(U), np.asarray(A),
        np.asarray(W1), np.asarray(b1), np.asarray(W2), np.asarray(b2),
    )

    key = (B_local, T)
    if key not in _PROGRAM_CACHE:
        _PROGRAM_CACHE[key] = _build_program(B_local, T)
    nc = _PROGRAM_CACHE[key]

    from concourse.bass_utils import run_bass_kernel_spmd

    in_maps = [
        {"Wk": Wk, "A2e": A2e, "Rt": np.ascontiguousarray(R[i])}
        for i in range(N_CORES)
    ]
    res = run_bass_kernel_spmd(nc, in_maps, core_ids=list(range(N_CORES)))
    out = np.empty((B, T), np.float32)
    for i in range(N_CORES):
        O1 = res.results[i]["O1"].reshape(T, B_local)  # [t, b]
        out[i * B_local : (i + 1) * B_local] = O1.T
    return out



# revision 8
# speedup vs baseline: 1.2733x; 1.2733x over previous
"""KAARMA-style multi-cell kernel recurrence on 8 Trainium2 NeuronCores.

Math reformulation (validated vs reference; see emu.py):
  per step t, per batch b, for every dictionary atom (c, m) [cm = 800 atoms]:
    phi[b,cm] = exp(-|s_b - S_cm|^2 - (x_tb - U_cm)^2)
    gate[b,c] = softmax_c(MLP(x_tb))                (precomputable, x-only)
    s'_b      = sum_cm gate[b,cell(cm)] * phi[b,cm] * A[cm,:]
  Expand the squares and fold the gate into the exponent:
    psi[b,cm] = exp( 2 s_b.S_cm - |s_b|^2 + 2 x U_cm + (lg[b,c] - x^2) )
    s'_b      = sum_cm psi[b,cm] * (A[cm,:] * exp(-|S_cm|^2 - U_cm^2))
  so one step = matmul([K,B] -> [896,B]) -> exp -> matmul([896,B] -> [16,B]).

All matmul operands are bf16 (1 PE cycle/row vs 4 for fp32); accumulation is
fp32 in PSUM. bf16 quantization of the per-batch-uniform exponent terms
(x^2, log-gate) would scale whole psi rows coherently, so those rows are
carried as hi/lo bf16 pairs (contraction rows are ~free: matmul cost is set
by output columns). Measured end-to-end rel err ~3e-3 (emu.py V1).

Contraction-row layout (engine SBUF writes must start at partition 0/32/64/96;
DMA writes have no such constraint):
  rows  0:16  s          (weights 2*S)        <- DVE copy from PSUM
  rows 32:48  s^2        (weights -1)         <- DVE square
  row  64     x_hi       (weights 2*U)        <- staged by DMA
  row  65     x_lo       (weights 2*U)
  rows 66:74  hi(lg - x^2) one-hot cell rows  (weights 1)
  rows 74:82  lo(lg - x^2) one-hot cell rows  (weights 1)
K = 82; gaps 16:32, 48:64 are zeroed once and have zero weight columns.

Latency structure: total time ~= T * L where L is the serial per-step chain
PE(mm1) -> ACT(exp) -> PE(mm2) -> DVE(copy, square) -> PE. The batch is split
into Q=4 independent chains of 16 columns, phase-staggered so engines
interleave them; emission is skewed (chain q's mm2/handoff ride two slots
behind its mm1/exp) so the in-order engines never wait inside another
chain's slot. Dependency tracking is effectively whole-tile, so every chain
gets its OWN rhs tiles and pool rotation keeps WAR deps two generations
stale (sharing one tile across chains serializes everything).

Per chain the state lives in two "tall" rhs tiles [82, STAGE*BH] (ping-pong
per 64-step stage): x/lg rows are DMA'd straight into partitions 64:82, the
DVE handoff writes s/s^2 at the next step's column, and row 0 (the permuted
output component) is DMA'd out once per stage -- no per-step memsets, copies,
or output extraction.

Sharding: pure data parallel, batch 512 -> 64 per core on 8 cores.
"""

import numpy as np

N_CORES = 8
CM = 800
CM_PAD = 896
NCHUNK = 7
KROWS = 82
NSTATE = 16
STAGE = 64  # recurrence steps per staging DMA
Q = 4       # independent batch chains per core

_PROGRAM_CACHE = {}

# sim-only experiment knobs (kernel() correctness path keeps defaults)
import os as _os
EXP_ENGINE = _os.environ.get("KEXP", "act")    # act | dve_fake
COPY_ENGINE = _os.environ.get("KCOPY", "dve")  # dve | pool
SQ_MODE = _os.environ.get("KSQ", "sbuf")       # sbuf | psum | pool


def _build_program(B_local, T):
    import concourse.bass as bass
    import concourse.bacc as bacc
    import concourse.tile as tile
    from concourse import mybir
    from contextlib import ExitStack

    f32 = mybir.dt.float32
    DT = mybir.dt.bfloat16
    Act = mybir.ActivationFunctionType

    BH = B_local // Q
    NST = T // STAGE  # number of stages
    SB = STAGE * BH   # columns per (stage, chain) block

    nc = bacc.Bacc("TRN2", target_bir_lowering=False, debug=False)
    W_d = nc.dram_tensor("Wk", [KROWS, CM_PAD], DT, kind="ExternalInput")
    A2_d = nc.dram_tensor("A2e", [CM_PAD, NSTATE], DT, kind="ExternalInput")
    R_d = nc.dram_tensor("Rt", [18, T * B_local], DT, kind="ExternalInput")
    O_d = nc.dram_tensor("O1", [1, T * B_local], DT, kind="ExternalOutput")

    with tile.TileContext(nc) as tc, ExitStack() as ctx:
        singles = ctx.enter_context(tc.tile_pool(name="singles", bufs=1))
        argpool = ctx.enter_context(
            tc.tile_pool(name="argp", bufs=Q, space="PSUM")
        )
        spool = ctx.enter_context(tc.tile_pool(name="sp", bufs=Q, space="PSUM"))
        psipool = ctx.enter_context(tc.tile_pool(name="psi", bufs=2 * Q))

        Wsb = singles.tile([KROWS, CM_PAD], DT)
        nc.sync.dma_start(out=Wsb, in_=W_d[:, :])
        A2sb = singles.tile([128, NCHUNK, NSTATE], DT)
        nc.sync.dma_start(out=A2sb, in_=A2_d.rearrange("(c p) n -> p c n", p=128))

        RH = [
            [singles.tile([KROWS, SB], DT, name=f"rh{i}_{q}") for q in range(Q)]
            for i in range(2)
        ]
        # zero the state/square rows + gaps once; gap rows stay zero forever
        for i in range(2):
            for q in range(Q):
                nc.gpsimd.memset(RH[i][q][0:64, :], 0.0)
        # stage 0 and 1 x/lg rows
        for si in range(min(2, NST)):
            for q in range(Q):
                nc.sync.dma_start(
                    out=RH[si][q][64:KROWS, :],
                    in_=R_d[:, (si * Q + q) * SB : (si * Q + q + 1) * SB],
                )

        sP = [None] * Q  # live state psum tile per chain
        psi = [None] * Q

        def rhs_ap(t, q):
            si, ti = divmod(t, STAGE)
            return RH[si % 2][q][0:KROWS, ti * BH : (ti + 1) * BH]

        def mm1(t, q, argt):
            r = rhs_ap(t, q)
            for k in range(NCHUNK):
                nc.tensor.matmul(
                    argt[:, k * BH : (k + 1) * BH],
                    lhsT=Wsb[:, k * 128 : (k + 1) * 128],
                    rhs=r,
                    start=True,
                    stop=True,
                )

        def mm2(t, q):
            p = psi[q]
            sP[q] = spool.tile([NSTATE, BH], f32, tag="sp", name=f"s{t}_{q}")
            for k in range(NCHUNK):
                nc.tensor.matmul(
                    sP[q],
                    lhsT=A2sb[:, k, :],
                    rhs=p[:, k * BH : (k + 1) * BH],
                    start=(k == 0),
                    stop=(k == NCHUNK - 1),
                    skip_group_check=True,
                )

        def handoff(t, q):
            # state for step t+1 into its rhs column slot
            nsi, nti = divmod(t + 1, STAGE)
            dst = RH[nsi % 2][q]
            c0 = nti * BH
            if COPY_ENGINE == "pool":
                nc.gpsimd.tensor_copy(
                    out=dst[0:NSTATE, c0 : c0 + BH], in_=sP[q]
                )
            else:
                nc.vector.tensor_scalar_add(
                    dst[0:NSTATE, c0 : c0 + BH], sP[q], 0.0
                )
            if SQ_MODE == "psum":
                nc.vector.tensor_mul(dst[32:48, c0 : c0 + BH], sP[q], sP[q])
            elif SQ_MODE == "pool":
                nc.gpsimd.tensor_mul(
                    dst[32:48, c0 : c0 + BH], sP[q], sP[q]
                )
            elif SQ_MODE == "pool_sbuf":
                nc.gpsimd.tensor_mul(
                    dst[32:48, c0 : c0 + BH],
                    dst[0:NSTATE, c0 : c0 + BH],
                    dst[0:NSTATE, c0 : c0 + BH],
                )
            else:
                nc.vector.tensor_mul(
                    dst[32:48, c0 : c0 + BH],
                    dst[0:NSTATE, c0 : c0 + BH],
                    dst[0:NSTATE, c0 : c0 + BH],
                )

        def boundary(si):
            # stage si-1's outputs: slot t+1 row 0 holds output[t]
            for q in range(Q):
                base = ((si - 1) * Q + q) * SB
                nc.sync.dma_start(
                    out=O_d[:, base : base + (STAGE - 1) * BH],
                    in_=RH[(si - 1) % 2][q][0:1, BH:SB],
                )
                nc.sync.dma_start(
                    out=O_d[:, base + (STAGE - 1) * BH : base + SB],
                    in_=RH[si % 2][q][0:1, 0:BH],
                )
                if si + 1 < NST:
                    nc.sync.dma_start(
                        out=RH[(si + 1) % 2][q][64:KROWS, :],
                        in_=R_d[:, ((si + 1) * Q + q) * SB : ((si + 1) * Q + q + 1) * SB],
                    )

        # skewed main loop: slot (t, q) emits chain q's mm1/exp for step t and
        # chain (q+HALF)%Q's mm2/handoff for its pending step
        HALF = Q // 2
        for t in range(T):
            for q in range(Q):
                argt = argpool.tile(
                    [128, NCHUNK * BH], f32, tag="argp", name=f"a{t}_{q}"
                )
                mm1(t, q, argt)
                psi[q] = psipool.tile(
                    [128, NCHUNK * BH], DT, tag="psi", name=f"p{t}_{q}"
                )
                if EXP_ENGINE == "dve_fake":
                    # timeline stand-in for a custom-DVE exp (sim only)
                    nc.vector.tensor_scalar_add(psi[q], argt, 0.0)
                else:
                    nc.scalar.activation(out=psi[q], in_=argt, func=Act.Exp)
                qp = (q + HALF) % Q
                tp = t - 1 if q < HALF else t
                if tp >= 0:
                    mm2(tp, qp)
                    handoff(tp, qp)
                if t % STAGE == 0 and t > 0 and q == HALF - 1:
                    boundary(t // STAGE)
        # epilogue: drain the last HALF chains' final mm2/handoff
        for q in range(HALF):
            qp = (q + HALF) % Q
            mm2(T - 1, qp)
            handoff(T - 1, qp)
        # final stage outputs
        for q in range(Q):
            base = ((NST - 1) * Q + q) * SB
            nc.sync.dma_start(
                out=O_d[:, base : base + (STAGE - 1) * BH],
                in_=RH[(NST - 1) % 2][q][0:1, BH:SB],
            )
            nc.sync.dma_start(
                out=O_d[:, base + (STAGE - 1) * BH : base + SB],
                in_=RH[NST % 2][q][0:1, 0:BH],
            )

    nc.compile()
    return nc


def _host_precompute(x, S, U, A, W1, b1, W2, b2):
    import ml_dtypes

    BF = np.dtype(ml_dtypes.bfloat16)
    B, T = x.shape
    C, M, N = S.shape
    B_local = B // N_CORES
    BH = B_local // Q

    # state permutation: put the output component (N-1) at row 0
    perm = np.r_[N - 1, np.arange(N - 1)]

    Sf = S.reshape(C * M, N).astype(np.float32)
    Uf = U.reshape(C * M).astype(np.float32)
    C1 = (Sf * Sf).sum(1) + Uf * Uf
    A2e = np.zeros((CM_PAD, N), np.float32)
    A2e[:CM] = (A.reshape(C * M, N) * np.exp(-C1)[:, None])[:, perm]

    Wk = np.zeros((KROWS, CM_PAD), np.float32)
    Wk[0:NSTATE, :CM] = 2.0 * Sf.T[perm]
    Wk[32:48, :CM] = -1.0
    Wk[64, :CM] = 2.0 * Uf
    Wk[65, :CM] = 2.0 * Uf
    for c in range(C):
        Wk[66 + c, c * M : (c + 1) * M] = 1.0
        Wk[74 + c, c * M : (c + 1) * M] = 1.0

    # gate log-softmax, x-only, fp32 on host
    h = np.maximum(x[..., None] * W1[0] + b1, 0.0)  # [B,T,16]
    g = h @ W2 + b2  # [B,T,C]
    g = g - g.max(-1, keepdims=True)
    lg = (g - np.log(np.exp(g).sum(-1, keepdims=True))).astype(np.float32)

    r = lg - (x * x)[..., None]  # [B,T,C]
    r_hi = r.astype(BF)
    r_lo = (r - r_hi.astype(np.float32)).astype(BF)
    x_hi = x.astype(BF)
    x_lo = (x - x_hi.astype(np.float32)).astype(BF)

    # per-core layout [18, NST, Q, STAGE, BH]: chain q owns batch cols
    # [q*BH, (q+1)*BH) of the core's 64
    NST = T // STAGE
    R = np.empty((N_CORES, 18, NST, Q, STAGE, BH), BF)
    for i in range(N_CORES):
        bs = slice(i * B_local, (i + 1) * B_local)
        # [T, B_local] -> [NST, STAGE, Q, BH] -> [NST, Q, STAGE, BH]
        def lay(a):  # a: [B, T]
            return (
                a[bs].T.reshape(NST, STAGE, Q, BH).transpose(0, 2, 1, 3)
            )
        R[i, 0] = lay(x_hi)
        R[i, 1] = lay(x_lo)
        for c in range(8):
            R[i, 2 + c] = lay(r_hi[:, :, c])
            R[i, 10 + c] = lay(r_lo[:, :, c])
    R = R.reshape(N_CORES, 18, T * B_local)
    return Wk.astype(BF), A2e.astype(BF), R


def kernel(x, S, U, A, W1, b1, W2, b2):
    x = np.asarray(x, np.float32)
    B, T = x.shape
    assert B % N_CORES == 0 and T % STAGE == 0
    B_local = B // N_CORES
    BH = B_local // Q

    Wk, A2e, R = _host_precompute(
        np.asarray(x), np.asarray(S), np.asarray(U), np.asarray(A),
        np.asarray(W1), np.asarray(b1), np.asarray(W2), np.asarray(b2),
    )

    key = (B_local, T)
    if key not in _PROGRAM_CACHE:
        _PROGRAM_CACHE[key] = _build_program(B_local, T)
    nc = _PROGRAM_CACHE[key]

    from concourse.bass_utils import run_bass_kernel_spmd

    in_maps = [
        {"Wk": Wk, "A2e": A2e, "Rt": np.ascontiguousarray(R[i])}
        for i in range(N_CORES)
    ]
    res = run_bass_kernel_spmd(nc, in_maps, core_ids=list(range(N_CORES)))
    NST = T // STAGE
    out = np.empty((B, T), np.float32)
    for i in range(N_CORES):
        O1 = res.results[i]["O1"].astype(np.float32).reshape(NST, Q, STAGE, BH)
        # -> [Q, BH, NST, STAGE] -> [B_local, T]
        o = O1.transpose(1, 3, 0, 2).reshape(B_local, T)
        out[i * B_local : (i + 1) * B_local] = o
    return out
